# revision 10
# baseline (speedup 1.0000x reference)
"""Ensemble-MLP (grouped 1x1 conv) Trainium2 kernel.

Computation (per batch row b):
  h = relu(x @ W0[e] + b0[e])             e = 0..9 ensembles, 256 units
  h = relu(h @ Wh[l,e] + bh[l,e])         l = 0..6 hidden layers
  y[e] = h @ Wf[e] + bf[e]                201 outputs per ensemble
  out[b, o'] = mean_j yflat[b, o'*10 + j] (strided channel mix, yflat = e*201+o)

Strategy:
  * Data parallel: batch 16384 -> 2048 rows per core on 8 cores. Weights
    replicated.
  * Activations live in SBUF transposed: H[channel, batch], channel on
    partitions (256 = 2 chunks of 128), batch on the free axis (2048).
  * Every layer is matmul(out[o, b] += W[c, o].T @ H[c, b]) in bf16
    (1 cycle/row on the PE like fp32r, but half the ldweights/DMA/SBUF
    traffic). PSUM accumulation is fp32; rel err ~1e-2 < 2e-2 gate.
  * Layer-0 bias is folded into the matmul via an all-ones row appended to
    x^T (K=7). Hidden biases ride the relu post-op (per-partition bias).
  * The final channel-mixing mean is folded into the last-layer weights on
    the host: out = sum_e H_e @ V[e] + bp, V[e] = Wf[e] @ Me[e] (exact
    linear algebra, no approximation).
  * PSUM tiles span 2 banks ([128, 1024] = two 512-col batch halves of
    the same output-channel chunk), so each relu/accumulate post-op
    covers 1024 columns with a single per-partition bias — half the
    post-op count and fixed overhead.
  * batch-pair-major issue order everywhere so each layer consumes h
    slices in the order the previous layer's post-ops retire them.
  * relu post-ops interleave 5:3 across ACT/DVE; final-layer accumulation
    on DVE. Output DMA'd slice-by-slice as the last ensemble's
    accumulation completes.
  * Dummy warm-up matmuls on a memset tile ramp the PE p-state during the
    initial DMA dead time (framework init blocks all queues until ~6us,
    so only ~7 fit before the weights land).
"""

import numpy as np
import ml_dtypes
from contextlib import ExitStack

import concourse.bass as bass
import concourse.mybir as mybir
import concourse.tile as tile
from concourse import bacc, bass_utils

F32 = mybir.dt.float32
DT = mybir.dt.bfloat16
NP_DT = ml_dtypes.bfloat16

ENS, N_UNITS, N_HID, IN_DIM, OUT_DIM, BATCH = 10, 256, 7, 6, 201, 16384
N_CORES = 8
BC = BATCH // N_CORES          # 2048 batch rows per core
NT = BC // 512                 # 4 moving-operand tiles of 512
NPAIR = BC // 1024             # 2 batch-pair tiles of 1024 (post-op grain)
N_WARM = 7                     # PE p-state warm-up matmuls

_CACHE = {}


def build_program():
    nc = bacc.Bacc("TRN2", debug=False)

    xt = nc.dram_tensor("xt", (IN_DIM + 1, BC), DT, kind="ExternalInput").ap()
    w0 = nc.dram_tensor("w0", (ENS, IN_DIM + 1, N_UNITS), DT, kind="ExternalInput").ap()
    wh = nc.dram_tensor("wh", (ENS, 128, N_HID * 2 * N_UNITS), DT, kind="ExternalInput").ap()
    bh = nc.dram_tensor("bh", (ENS, 128, N_HID * 2), F32, kind="ExternalInput").ap()
    vw = nc.dram_tensor("vw", (ENS, 128, 2 * 256), DT, kind="ExternalInput").ap()
    bp = nc.dram_tensor("bp", (128, 2), F32, kind="ExternalInput").ap()
    yt = nc.dram_tensor("yt", (256, BC), F32, kind="ExternalOutput").ap()

    add = mybir.AluOpType.add
    mx = mybir.AluOpType.max
    relu = mybir.ActivationFunctionType.Relu

    with ExitStack() as ctx:
        tc = ctx.enter_context(tile.TileContext(nc))
        const = ctx.enter_context(tc.tile_pool(name="const", bufs=1))
        wpool = ctx.enter_context(tc.tile_pool(name="w", bufs=2))
        hpool = ctx.enter_context(tc.tile_pool(name="h", bufs=2))
        opool = ctx.enter_context(tc.tile_pool(name="acc", bufs=1))
        pspool = ctx.enter_context(tc.tile_pool(name="ps", bufs=4, space="PSUM"))

        x_t = const.tile([IN_DIM + 1, BC], DT)
        bp_t = const.tile([128, 2], F32)
        warm = const.tile([128, 512], DT)
        out_t = [opool.tile([128, BC], F32, tag=f"out{i}", name=f"out{i}")
                 for i in range(2)]

        # PE p-state warm-up: matmuls on a zeroed tile into the PSUM
        # rotation, covering the initial DMA dead time.
        nc.gpsimd.memset(warm[:, :], 0.0)
        for _ in range(N_WARM):
            ps = pspool.tile([128, 1024], F32, tag="ps")
            nc.tensor.matmul(ps[:, 0:512], lhsT=warm[:, 0:128], rhs=warm,
                             start=True, stop=True)

        post_i = 0  # relu post-ops interleaved 5:3 across ACT/DVE
        # (GpSimd/Pool cannot access PSUM on this toolchain; DVE also
        # carries the final-layer accumulates)

        def relu_post(dst, ps, bias_ap):
            nonlocal post_i
            use_act = post_i % 8 in (0, 2, 3, 5, 6)
            post_i += 1
            if use_act:
                nc.scalar.activation(out=dst, in_=ps, func=relu,
                                     bias=bias_ap if bias_ap is not None else 0.0)
            elif bias_ap is not None:
                nc.vector.tensor_scalar(out=dst, in0=ps, scalar1=bias_ap,
                                        scalar2=0.0, op0=add, op1=mx)
            else:
                nc.vector.tensor_scalar(out=dst, in0=ps, scalar1=0.0,
                                        scalar2=None, op0=mx)

        for e in range(ENS):
            if e == 0:
                # x first: layer-0's first matmuls wait only on their slice.
                for bt in range(NT):
                    nc.sync.dma_start(out=x_t[:, bt * 512:(bt + 1) * 512],
                                      in_=xt[:, bt * 512:(bt + 1) * 512])
            w0_t = wpool.tile([IN_DIM + 1, N_UNITS], DT, tag="w0")
            nc.sync.dma_start(out=w0_t, in_=w0[e])
            if e == 0:
                nc.sync.dma_start(out=bp_t, in_=bp)
            wh_t = wpool.tile([128, N_HID * 2 * N_UNITS], DT, tag="wh")
            # two transfers: first two layers' weights land early
            nc.sync.dma_start(out=wh_t[:, :1024], in_=wh[e][:, :1024])
            nc.sync.dma_start(out=wh_t[:, 1024:], in_=wh[e][:, 1024:])
            bh_t = wpool.tile([128, N_HID * 2], F32, tag="bh")
            nc.sync.dma_start(out=bh_t, in_=bh[e])
            v_t = wpool.tile([128, 2 * 256], DT, tag="v")
            nc.sync.dma_start(out=v_t, in_=vw[e])

            # h tiles: one [128, BC] tile per output-channel chunk; each
            # post-op fills a 1024-col slice (slice-granular deps).
            h_cur = [hpool.tile([128, BC], DT, tag=f"h{oc}", name=f"h{oc}_e{e}")
                     for oc in range(2)]

            # ---- layer 0: x^T (7, BC) -> h (2x128, BC); bias folded in ----
            for bp_i in range(NPAIR):
                cols = slice(bp_i * 1024, (bp_i + 1) * 1024)
                for oc in range(2):
                    ps = pspool.tile([128, 1024], F32, tag="ps")
                    for hf in range(2):
                        sl = slice(bp_i * 1024 + hf * 512,
                                   bp_i * 1024 + hf * 512 + 512)
                        nc.tensor.matmul(ps[:, hf * 512:(hf + 1) * 512],
                                         lhsT=(w0_t[:, oc * 128:(oc + 1) * 128]),
                                         rhs=(x_t[:, sl]), start=True, stop=True)
                    relu_post(h_cur[oc][:, cols], ps, None)

            # ---- 7 hidden layers: K=256 (2 chunks), M=256 (2 chunks) ----
            for l in range(N_HID):
                h_nxt = [hpool.tile([128, BC], DT, tag=f"h{oc}",
                                    name=f"h{oc}_e{e}l{l}") for oc in range(2)]
                base = l * 2 * N_UNITS
                for bp_i in range(NPAIR):
                    cols = slice(bp_i * 1024, (bp_i + 1) * 1024)
                    for oc in range(2):
                        ps = pspool.tile([128, 1024], F32, tag="ps")
                        for hf in range(2):
                            sl = slice(bp_i * 1024 + hf * 512,
                                       bp_i * 1024 + hf * 512 + 512)
                            psl = ps[:, hf * 512:(hf + 1) * 512]
                            nc.tensor.matmul(
                                psl, lhsT=(wh_t[:, base + oc * 128: base + oc * 128 + 128]),
                                rhs=(h_cur[0][:, sl]), start=True, stop=False)
                            nc.tensor.matmul(
                                psl, lhsT=(wh_t[:, base + N_UNITS + oc * 128: base + N_UNITS + oc * 128 + 128]),
                                rhs=(h_cur[1][:, sl]), start=False, stop=True)
                        relu_post(h_nxt[oc][:, cols], ps,
                                  bh_t[:, l * 2 + oc: l * 2 + oc + 1])
                h_cur = h_nxt

            # ---- final layer: out[o', b] += sum_kc V[e][kc].T @ h[kc] ----
            # o' padded to 256 so both output chunks are full 128-partition
            # tiles. The last ensemble's accumulate triggers the output DMA
            # for its slice.
            for bp_i in range(NPAIR):
                cols = slice(bp_i * 1024, (bp_i + 1) * 1024)
                for oc in range(2):
                    ps = pspool.tile([128, 1024], F32, tag="ps")
                    for hf in range(2):
                        sl = slice(bp_i * 1024 + hf * 512,
                                   bp_i * 1024 + hf * 512 + 512)
                        psl = ps[:, hf * 512:(hf + 1) * 512]
                        nc.tensor.matmul(psl, lhsT=(v_t[:, oc * 128: oc * 128 + 128]),
                                         rhs=(h_cur[0][:, sl]), start=True, stop=False)
                        nc.tensor.matmul(psl, lhsT=(v_t[:, 256 + oc * 128: 256 + oc * 128 + 128]),
                                         rhs=(h_cur[1][:, sl]), start=False, stop=True)
                    if e == 0:
                        nc.vector.tensor_scalar(out=out_t[oc][:, cols], in0=ps,
                                                scalar1=bp_t[:, oc:oc + 1],
                                                scalar2=None, op0=add)
                    else:
                        nc.vector.tensor_tensor(out=out_t[oc][:, cols],
                                                in0=out_t[oc][:, cols],
                                                in1=ps, op=add)
                    if e == ENS - 1:
                        nc.sync.dma_start(out=yt[oc * 128:(oc + 1) * 128, cols],
                                          in_=out_t[oc][:, cols])

    nc.compile()
    return nc


def prepare_inputs(x, W0, b0, Wh, bh, Wf, bf):
    """Host-side weight refactoring + per-core sharding. Exact fp32
    linear algebra (bias folds + the channel-mix mean folded into Wf),
    then bf16 rounding of the matmul operands."""
    x = np.asarray(x, np.float32)
    W0 = np.asarray(W0, np.float32)
    b0 = np.asarray(b0, np.float32)
    Wh = np.asarray(Wh, np.float32)
    bh = np.asarray(bh, np.float32)
    Wf = np.asarray(Wf, np.float32)
    bf = np.asarray(bf, np.float32)

    # layer 0 with bias folded: lhsT rows = 6 inputs + ones row
    w0a = np.concatenate([W0, b0[:, None, :]], axis=1)  # (ENS, 7, 256)
    w0a = np.ascontiguousarray(w0a).astype(NP_DT)

    # hidden weights -> [e, p, (l, kc, o)]
    whh = (Wh.transpose(1, 0, 2, 3)              # (e, l, h, o)
             .reshape(ENS, N_HID, 2, 128, N_UNITS)
             .transpose(0, 3, 1, 2, 4)           # (e, p, l, kc, o)
             .reshape(ENS, 128, N_HID * 2 * N_UNITS))
    whh = np.ascontiguousarray(whh).astype(NP_DT)

    # hidden biases -> [e, p, (l, oc)]
    bhh = (bh.transpose(1, 0, 2)                 # (e, l, o)
             .reshape(ENS, N_HID, 2, 128)
             .transpose(0, 3, 1, 2)              # (e, p, l, oc)
             .reshape(ENS, 128, N_HID * 2))
    bhh = np.ascontiguousarray(bhh)

    # fold the strided channel-mix mean into the final weights:
    # out[b, o'] = 0.1 * sum_j yflat[b, o'*10+j],  yflat col c = e*201+o
    C = ENS * OUT_DIM
    M = np.zeros((C, OUT_DIM), np.float32)
    M[np.arange(C), np.arange(C) // ENS] = 1.0 / ENS
    Me = M.reshape(ENS, OUT_DIM, OUT_DIM)
    V = np.einsum('eho,eoc->ehc', Wf, Me)        # (ENS, 256, 201)
    bpv = bf.reshape(C) @ M                      # (201,)

    Vp = np.zeros((ENS, N_UNITS, 256), np.float32)
    Vp[:, :, :OUT_DIM] = V
    vww = (Vp.reshape(ENS, 2, 128, 256)
             .transpose(0, 2, 1, 3)              # (e, p, kc, o')
             .reshape(ENS, 128, 2 * 256))
    vww = np.ascontiguousarray(vww).astype(NP_DT)

    bp_pad = np.zeros(256, np.float32)
    bp_pad[:OUT_DIM] = bpv
    bp_t = np.ascontiguousarray(bp_pad.reshape(2, 128).T)  # (128, 2)

    ones = np.ones((1, BC), np.float32)
    in_maps = []
    for c in range(N_CORES):
        xs = x[c * BC:(c + 1) * BC]              # (BC, 6)
        xt = np.ascontiguousarray(
            np.concatenate([xs.T, ones], axis=0)).astype(NP_DT)  # (7, BC)
        in_maps.append({
            "xt": xt, "w0": w0a, "wh": whh, "bh": bhh, "vw": vww, "bp": bp_t,
        })
    return in_maps


def run(in_maps, trace=False, tmpdir=None):
    if "nc" not in _CACHE:
        _CACHE["nc"] = build_program()
    nc = _CACHE["nc"]
    res = bass_utils.run_bass_kernel_spmd(
        nc, in_maps, core_ids=list(range(N_CORES)), trace=trace, tmpdir=tmpdir)
    return res


def kernel(x, W0, b0, Wh, bh, Wf, bf):
    in_maps = prepare_inputs(x, W0, b0, Wh, bh, Wf, bf)
    res = run(in_maps)
    out = np.empty((BATCH, OUT_DIM), np.float32)
    for c in range(N_CORES):
        out[c * BC:(c + 1) * BC, :] = res.results[c]["yt"][:OUT_DIM].T
    return out
